# revision 57
# baseline (speedup 1.0000x reference)
"""Original baseline kernel (v1) for A/B re-measurement in today's power env."""

import os
import sys

import numpy as np

for _p in ("/opt/trn_rl_repo",):
    if os.path.isdir(_p) and _p not in sys.path:
        sys.path.insert(0, _p)

import ml_dtypes

T = 4096
C = 1024
H = 16
DH = 64
R = 8
HL = H // R
CH = C // R
QW = 512
KB = 128
NQW = T // QW
NKB = T // KB
NCH = C // 128
HB = QW // 2    # half-window column split for the last window's gather
SCALE = float(C) ** -0.5
BF16 = ml_dtypes.bfloat16
OPROJ_AT = {5: [0], 6: [1, 2], 7: [3, 4, 5]}
# gather-output loads are issued on the gpsimd ring, which also carries the
# normalize broadcasts; a load issued before its gather completed stalls the
# whole queue (the ring only opens at ~70us because of the cross-core startup
# barrier), so each load is scheduled a window later than its gather's
# completion under worst-case ring backlog.
FL_AT_END = {3: [0], 4: [1], 5: [2], 6: [3, 4, 5]}
FL_AT_START = {7: [6]}

LAST_RESULT = None

_nc = None


def _build():
    import concourse.mybir as mybir
    import concourse.tile as tile
    from concourse import bacc

    f32 = mybir.dt.float32
    bf16 = mybir.dt.bfloat16
    EXP = mybir.ActivationFunctionType.Exp

    nc = bacc.Bacc("TRN2", target_bir_lowering=False, num_devices=R)

    xT_d = nc.declare_dram_parameter("xT", [C, T], bf16, isOutput=False)
    wq_d = nc.declare_dram_parameter("wqT", [128, NCH * CH], bf16, isOutput=False)
    wk_d = nc.declare_dram_parameter("wkT", [128, NCH * CH], bf16, isOutput=False)
    wv_d = nc.declare_dram_parameter("wvT", [128, NCH * CH], bf16, isOutput=False)
    wp_d = nc.declare_dram_parameter("wpT", [128, NCH * CH], bf16, isOutput=False)
    bq_d = nc.declare_dram_parameter("bqc", [CH, 1], f32, isOutput=False)
    bk_d = nc.declare_dram_parameter("bkc", [CH, 1], f32, isOutput=False)
    bv_d = nc.declare_dram_parameter("bvc", [CH, 1], f32, isOutput=False)
    bp_d = nc.declare_dram_parameter("bpc", [CH, 1], f32, isOutput=False)
    cm_d = nc.declare_dram_parameter("cmask", [128, 4 * QW], bf16, isOutput=False)
    id_d = nc.declare_dram_parameter("ident", [128, 128], bf16, isOutput=False)
    out_d = nc.declare_dram_parameter("out", [CH, T], f32, isOutput=True)

    with tile.TileContext(nc, num_cores=R) as tc:
        with (
            tc.tile_pool(name="const", bufs=1) as constp,
            tc.tile_pool(name="big", bufs=1) as bigp,
            tc.tile_pool(name="dram", bufs=1, space="DRAM") as dramp,
        ):
            xs = bigp.tile([128, NCH * T], bf16)
            qt_s = bigp.tile([128, T], bf16)
            kt_s = bigp.tile([128, T], bf16)
            vb_s = bigp.tile([128, HL * NKB * 65], bf16)
            wq_s = constp.tile([128, NCH * CH], bf16)
            wk_s = constp.tile([128, NCH * CH], bf16)
            wv_s = constp.tile([128, NCH * CH], bf16)
            wp_s = constp.tile([128, NCH * CH], bf16)
            bq_s = constp.tile([CH, 1], f32)
            bk_s = constp.tile([CH, 1], f32)
            bv_s = constp.tile([CH, 1], f32)
            bp_s = constp.tile([CH, 1], f32)
            cm_s = constp.tile([128, 4 * QW], bf16)
            ident = constp.tile([128, 128], bf16)

            nc.sync.dma_start(ident[:], id_d[:])
            nc.sync.dma_start(wq_s[:], wq_d[:])
            nc.sync.dma_start(wk_s[:], wk_d[:])

            xs3 = xs[:].rearrange("p (c t) -> p c t", c=NCH)
            xT3 = xT_d[:].rearrange("(c p) t -> p c t", c=NCH)

            def load_x_eighth(tq):
                nc.sync.dma_start(
                    xs3[:, :, tq * QW:(tq + 1) * QW],
                    xT3[:, :, tq * QW:(tq + 1) * QW],
                )

            nc.sync.dma_start(xs3[:, 0:4, 0:QW], xT3[:, 0:4, 0:QW])
            nc.scalar.dma_start(xs3[:, 4:8, 0:QW], xT3[:, 4:8, 0:QW])
            for b_s, b_d in ((bq_s, bq_d), (bk_s, bk_d), (bv_s, bv_d), (bp_s, bp_d)):
                nc.sync.dma_start(b_s[:], b_d[:])
            nc.sync.dma_start(wv_s[:], wv_d[:])
            load_x_eighth(1)
            nc.sync.dma_start(cm_s[:], cm_d[:])
            nc.sync.dma_start(wp_s[:], wp_d[:])
            load_x_eighth(2)

            nc.gpsimd.memset(vb_s[:], 1.0)

            gouts = [None] * NQW
            fls = [None] * NQW
            with (
                tc.tile_pool(name="stp", bufs=2, space="PSUM") as stp,
                tc.tile_pool(name="otp", bufs=2, space="PSUM") as otp,
                tc.tile_pool(name="vap", bufs=2, space="PSUM") as vap,
                tc.tile_pool(name="pp", bufs=8) as pp,
                tc.tile_pool(name="aop", bufs=2) as aop,
                tc.tile_pool(name="vtp", bufs=2) as vtp,
                tc.tile_pool(name="flp", bufs=4) as flp,
                tc.tile_pool(name="fop", bufs=2) as fop,
                tc.tile_pool(name="smallp", bufs=4) as smallp,
            ):
                def make_qk_groups(tw):
                    groups = []
                    for w_s, b_s, dst, nm in (
                        (wq_s, bq_s, qt_s, "q"),
                        (wk_s, bk_s, kt_s, "k"),
                    ):
                        box = {}

                        def g0(w_s=w_s, box=box, tw=tw, nm=nm):
                            acc = vap.tile(
                                [128, QW], f32, tag="vacc", name=f"acc{nm}{tw}"
                            )
                            box["acc"] = acc
                            for c in range(4):
                                nc.tensor.matmul(
                                    acc[:],
                                    w_s[:, c * CH:(c + 1) * CH],
                                    xs[:, c * T + tw * QW: c * T + tw * QW + QW],
                                    start=(c == 0),
                                    stop=False,
                                    skip_group_check=True,
                                )

                        def g1(w_s=w_s, b_s=b_s, dst=dst, box=box, tw=tw):
                            acc = box["acc"]
                            for c in range(4, NCH):
                                nc.tensor.matmul(
                                    acc[:],
                                    w_s[:, c * CH:(c + 1) * CH],
                                    xs[:, c * T + tw * QW: c * T + tw * QW + QW],
                                    start=False,
                                    stop=(c == NCH - 1),
                                    skip_group_check=True,
                                )
                            nc.vector.tensor_scalar_add(
                                dst[:, tw * QW:(tw + 1) * QW], acc[:], b_s[:]
                            )

                        groups += [g0, g1]
                    return groups

                def make_v_groups(tw):
                    box = {}

                    def g0():
                        acc = vap.tile([128, QW], f32, tag="vacc", name=f"vacc{tw}")
                        box["acc"] = acc
                        for c in range(4):
                            nc.tensor.matmul(
                                acc[:],
                                wv_s[:, c * CH:(c + 1) * CH],
                                xs[:, c * T + tw * QW: c * T + tw * QW + QW],
                                start=(c == 0),
                                stop=False,
                                skip_group_check=True,
                            )

                    def g1(tw=tw):
                        acc = box["acc"]
                        for c in range(4, NCH):
                            nc.tensor.matmul(
                                acc[:],
                                wv_s[:, c * CH:(c + 1) * CH],
                                xs[:, c * T + tw * QW: c * T + tw * QW + QW],
                                start=False,
                                stop=(c == NCH - 1),
                                skip_group_check=True,
                            )
                        vt = vtp.tile([128, QW], bf16, tag="vt")
                        box["vt"] = vt
                        nc.vector.tensor_scalar_add(vt[:], acc[:], bv_s[:])

                    def g2(tw=tw):
                        vt = box["vt"]
                        tp = vap.tile([128, QW], bf16, tag="vacc", name=f"tp{tw}")
                        for j in range(4):
                            nc.tensor.transpose(
                                tp[:, j * 128:(j + 1) * 128],
                                vt[:, j * 128:(j + 1) * 128],
                                ident[:],
                            )
                        # one strided copy per head moves all 4 token blocks
                        # into the 65-stride V layout (vs 8 small copies)
                        tp3 = tp[:].rearrange("p (j c) -> p j c", j=4)
                        vb3 = vb_s[:].rearrange("p (n c) -> p n c", c=65)
                        for h in range(HL):
                            nc.vector.tensor_copy(
                                vb3[:, h * NKB + tw * 4: h * NKB + tw * 4 + 4, 0:64],
                                tp3[:, :, h * 64: h * 64 + 64],
                            )

                    return [g0, g1, g2]

                def emit_fl(p, split=False):
                    fl = flp.tile([128, NCH * QW], bf16, tag="fl", name=f"fl{p}")
                    fls[p] = fl
                    fl3 = fl[:].rearrange("p (c m) -> p c m", c=NCH)
                    go3 = gouts[p][:].rearrange("(c p) m -> p c m", c=NCH)
                    if split:
                        nc.gpsimd.dma_start(fl3[:, 0:3], go3[:, 0:3])
                        nc.sync.dma_start(fl3[:, 3:6], go3[:, 3:6])
                        nc.scalar.dma_start(fl3[:, 6:8], go3[:, 6:8])
                    else:
                        nc.gpsimd.dma_start(fl3, go3)

                def emit_oproj(p):
                    fl = fls[p]
                    po = vap.tile([128, QW], f32, tag="vacc", name=f"po{p}")
                    for c in range(NCH):
                        nc.tensor.matmul(
                            po[:],
                            wp_s[:, c * CH:(c + 1) * CH],
                            fl[:, c * QW:(c + 1) * QW],
                            start=(c == 0),
                            stop=(c == NCH - 1),
                        )
                    fo = fop.tile([128, QW], f32, tag="fo")
                    nc.vector.tensor_scalar_add(fo[:], po[:], bp_s[:])
                    nc.sync.dma_start(out_d[:, p * QW:(p + 1) * QW], fo[:])

                with tc.high_priority():
                    prime = smallp.tile([CH, 1], f32, tag="prime")
                    nc.scalar.activation(
                        prime[:], ident[:, 0:1], EXP, bias=0.0, scale=0.0
                    )
                    warm = vap.tile([128, QW], f32, tag="vacc", name="warm")
                    for i in range(4):
                        nc.tensor.matmul(
                            warm[:, 0:128], ident[:], ident[:],
                            start=(i == 0), stop=(i == 3), skip_group_check=True,
                        )

                for g in make_qk_groups(0):
                    g()
                for g in make_v_groups(0):
                    g()

                for qw in range(NQW):
                    if qw + 3 < NQW:
                        load_x_eighth(qw + 3)
                    for p in FL_AT_START.get(qw, ()):
                        emit_fl(p)
                    pending = []
                    if qw + 1 < NQW:
                        pending += make_qk_groups(qw + 1)
                        pending += make_v_groups(qw + 1)
                    for p in OPROJ_AT.get(qw, ()):
                        pending.append(lambda p=p: emit_oproj(p))
                    npend0 = max(1, len(pending))
                    nkb = 4 * (qw + 1)
                    npairs = nkb // 2
                    ots = [
                        otp.tile([65, QW], f32, tag="ot", name=f"ot{qw}_{h}")
                        for h in range(HL)
                    ]
                    ao = aop.tile([128, QW], bf16, tag="ao")

                    def norm_half(lo, hi, sfx):
                        # normalize ao columns [lo:hi) out of the ot PSUM.
                        # For the last window this runs for cols [0:HB) right
                        # after pair npairs-2 — the PV writes to those
                        # columns are final then (the remaining diagonal
                        # blocks only touch cols >= HB), so the first half
                        # gather overlaps the last pair and the tail.
                        w = hi - lo
                        recs = []
                        for h in range(HL):
                            den = smallp.tile(
                                [1, w], f32, tag=f"dh{sfx}", name=f"dh{h}{sfx}"
                            )
                            nc.vector.tensor_copy(den[:], ots[h][64:65, lo:hi])
                            rec = smallp.tile(
                                [1, w], f32, tag=f"rh{sfx}", name=f"rh{h}{sfx}"
                            )
                            nc.vector.reciprocal_approx_fast(rec[:], den[:])
                            recs.append(rec)
                        for h in range(HL):
                            rb = smallp.tile([64, w], f32, tag=f"rbh{sfx}")
                            nc.gpsimd.partition_broadcast(rb[:], recs[h][:])
                            nc.vector.tensor_mul(
                                ao[h * 64:(h + 1) * 64, lo:hi],
                                ots[h][0:64, lo:hi],
                                rb[:],
                            )

                    def gather_half(lo, hi, sfx):
                        w = hi - lo
                        ginh = dramp.tile([128, w], bf16, tag=f"gin7{sfx}")
                        for h in range(HL):
                            nc.sync.dma_start(
                                ginh[h * 64:(h + 1) * 64, :],
                                ao[h * 64:(h + 1) * 64, lo:hi],
                            )
                        gouth = dramp.tile(
                            [R * 128, w], bf16, tag=f"gout7{sfx}",
                            addr_space="Shared",
                        )
                        nc.gpsimd.collective_compute(
                            "AllGather",
                            mybir.AluOpType.bypass,
                            ins=[ginh.opt()],
                            outs=[gouth.opt()],
                            replica_groups=[list(range(R))],
                        )
                        return gouth

                    for pair_i, kb0 in enumerate(range(0, nkb, 2)):
                        kbs = (kb0, kb0 + 1)
                        q0s = [max(0, kb * KB - qw * QW) for kb in kbs]
                        sts = []
                        for kb, q0 in zip(kbs, q0s):
                            st = stp.tile([128, 2 * QW], f32, tag="st")
                            for h in range(HL):
                                nc.tensor.matmul(
                                    st[:, h * QW + q0:(h + 1) * QW],
                                    kt_s[h * 64:(h + 1) * 64, kb * KB:(kb + 1) * KB],
                                    qt_s[h * 64:(h + 1) * 64,
                                         qw * QW + q0:(qw + 1) * QW],
                                    start=True,
                                    stop=True,
                                )
                            sts.append(st)
                        ps = []
                        for st, kb, q0 in zip(sts, kbs, q0s):
                            p = pp.tile([128, 2 * QW], bf16, tag="p")
                            if q0 >= 256:
                                for h in range(HL):
                                    nc.scalar.activation(
                                        p[:, h * QW + q0:(h + 1) * QW],
                                        st[:, h * QW + q0:(h + 1) * QW],
                                        EXP, bias=0.0, scale=SCALE,
                                    )
                                j = (kb * KB - qw * QW) // KB
                                for h in range(HL):
                                    nc.vector.tensor_mul(
                                        p[:, h * QW + q0:(h + 1) * QW],
                                        p[:, h * QW + q0:(h + 1) * QW],
                                        cm_s[:, j * QW + q0:(j + 1) * QW],
                                    )
                            elif q0 > 0:
                                nc.scalar.activation(
                                    p[:], st[:], EXP, bias=0.0, scale=SCALE
                                )
                                j = (kb * KB - qw * QW) // KB
                                for h in range(HL):
                                    nc.vector.tensor_mul(
                                        p[:, h * QW + q0:(h + 1) * QW],
                                        p[:, h * QW + q0:(h + 1) * QW],
                                        cm_s[:, j * QW + q0:(j + 1) * QW],
                                    )
                            else:
                                nc.scalar.activation(
                                    p[:], st[:], EXP, bias=0.0, scale=SCALE
                                )
                                if kb * KB == qw * QW:
                                    for h in range(HL):
                                        nc.vector.tensor_mul(
                                            p[:, h * QW:(h + 1) * QW],
                                            p[:, h * QW:(h + 1) * QW],
                                            cm_s[:, 0:QW],
                                        )
                            ps.append(p)
                        for p, kb, q0 in zip(ps, kbs, q0s):
                            for h in range(HL):
                                base = (h * NKB + kb) * 65
                                nc.tensor.matmul(
                                    ots[h][:, q0:QW],
                                    vb_s[:, base:base + 65],
                                    p[:, h * QW + q0:(h + 1) * QW],
                                    start=(kb == 0),
                                    stop=(kb == nkb - 1),
                                    skip_group_check=True,
                                )
                        # spread pending groups over the pairs, holding one
                        # back for the window boundary: the next window's
                        # first PV waits on this window's normalize (ot PSUM
                        # slot recycle), so the PE needs filler right after
                        # the last pair.
                        want_left = (npairs - 1 - pair_i) * npend0 // npairs
                        if pending:
                            want_left = max(1, want_left)
                        while pending and len(pending) > want_left:
                            pending.pop(0)()
                        if qw == NQW - 1 and pair_i == npairs - 2:
                            norm_half(0, HB, "a")
                            g7a = gather_half(0, HB, "a")
                    while pending:
                        pending.pop(0)()
                    if qw < NQW - 1:
                        norm_half(0, QW, "f")
                        gin = dramp.tile([128, QW], bf16, tag=f"gin{qw}")
                        for h in range(HL):
                            nc.sync.dma_start(
                                gin[h * 64:(h + 1) * 64, :],
                                ao[h * 64:(h + 1) * 64, :],
                            )
                        gout = dramp.tile(
                            [R * 128, QW], bf16, tag=f"gout{qw}",
                            addr_space="Shared",
                        )
                        nc.gpsimd.collective_compute(
                            "AllGather",
                            mybir.AluOpType.bypass,
                            ins=[gin.opt()],
                            outs=[gout.opt()],
                            replica_groups=[list(range(R))],
                        )
                        gouts[qw] = gout
                    else:
                        norm_half(HB, QW, "b")
                        g7b = gather_half(HB, QW, "b")
                    for p in FL_AT_END.get(qw, ()):
                        emit_fl(p)

                # tail: half-a of window 7 gathered early (mid-window); its
                # load + projection run while half-b's gather is on the ring.
                def tail_half(gouth, lo, hi, sfx):
                    w = hi - lo
                    flh = flp.tile(
                        [128, NCH * w], bf16, tag="fl", name=f"fl7{sfx}"
                    )
                    fh3 = flh[:].rearrange("p (c m) -> p c m", c=NCH)
                    gh3 = gouth[:].rearrange("(c p) m -> p c m", c=NCH)
                    nc.gpsimd.dma_start(fh3[:, 0:3], gh3[:, 0:3])
                    nc.sync.dma_start(fh3[:, 3:6], gh3[:, 3:6])
                    nc.scalar.dma_start(fh3[:, 6:8], gh3[:, 6:8])
                    return flh

                def proj_half(flh, lo, hi, sfx):
                    w = hi - lo
                    po = vap.tile([128, w], f32, tag="vacc", name=f"po7{sfx}")
                    for c in range(NCH):
                        nc.tensor.matmul(
                            po[:],
                            wp_s[:, c * CH:(c + 1) * CH],
                            flh[:, c * w:(c + 1) * w],
                            start=(c == 0),
                            stop=(c == NCH - 1),
                        )
                    fo = fop.tile([128, w], f32, tag="fo", name=f"fo7{sfx}")
                    nc.vector.tensor_scalar_add(fo[:], po[:], bp_s[:])
                    nc.sync.dma_start(
                        out_d[:, (NQW - 1) * QW + lo:(NQW - 1) * QW + hi], fo[:]
                    )

                fl7a = tail_half(g7a, 0, HB, "a")
                emit_oproj(6)
                proj_half(fl7a, 0, HB, "a")
                fl7b = tail_half(g7b, HB, QW, "b")
                proj_half(fl7b, HB, QW, "b")

    nc.compile()
    return nc


def _get_nc():
    global _nc
    if _nc is None:
        _nc = _build()
    return _nc


def _chunked_wT(w):
    wt = np.ascontiguousarray(w.T).reshape(NCH, 128, CH)
    return np.ascontiguousarray(
        wt.transpose(1, 0, 2).reshape(128, NCH * CH)
    ).astype(BF16)


def _causal_masks():
    kl = np.arange(KB)[:, None]
    ql = np.arange(QW)[None, :]
    cols = []
    for j in range(4):
        cols.append((kl + j * KB <= ql).astype(np.float32))
    return np.concatenate(cols, axis=1).astype(BF16)


def kernel(x, Wq, bq, Wk, bk, Wv, bv, Wp, bp):
    global LAST_RESULT
    from concourse.bass_utils import run_bass_kernel_spmd

    x = np.asarray(x, np.float32)
    Wq = np.asarray(Wq, np.float32)
    Wk = np.asarray(Wk, np.float32)
    Wv = np.asarray(Wv, np.float32)
    Wp = np.asarray(Wp, np.float32)
    bq = np.asarray(bq, np.float32)
    bk = np.asarray(bk, np.float32)
    bv = np.asarray(bv, np.float32)
    bp = np.asarray(bp, np.float32)

    xT16 = np.ascontiguousarray(x.T).astype(BF16)
    cmask = _causal_masks()
    ident = np.eye(128, dtype=np.float32).astype(BF16)

    in_maps = []
    for r in range(R):
        sl = slice(r * CH, (r + 1) * CH)
        in_maps.append(
            {
                "xT": xT16,
                "wqT": _chunked_wT(Wq[sl, :]),
                "wkT": _chunked_wT(Wk[sl, :]),
                "wvT": _chunked_wT(Wv[sl, :]),
                "wpT": _chunked_wT(Wp[sl, :]),
                "bqc": np.ascontiguousarray(bq[sl][:, None]),
                "bkc": np.ascontiguousarray(bk[sl][:, None]),
                "bvc": np.ascontiguousarray(bv[sl][:, None]),
                "bpc": np.ascontiguousarray(bp[sl][:, None]),
                "cmask": cmask,
                "ident": ident,
            }
        )

    nc = _get_nc()
    res = run_bass_kernel_spmd(nc, in_maps, core_ids=list(range(R)))
    LAST_RESULT = res
    out = np.empty((T, C), np.float32)
    for r in range(R):
        out[:, r * CH:(r + 1) * CH] = np.asarray(
            res.results[r]["out"], np.float32
        ).T
    return out
